# revision 5
# baseline (speedup 1.0000x reference)
"""TRN2 Bass kernel for nn_AttentionMP (GNN message passing attention).

Row-parallel attention across 8 NeuronCores: core c owns query rows
[c*1024, (c+1)*1024). Scores are computed TRANSPOSED, sT[j, i] (j = key
index on partitions, i = this core's query rows on the free dim), so
att^T feeds downstream matmuls directly.

v2 redesign: the per-j-tile work is spread across ALL engines so each
lands just under the ACT exp floor (~63us):
 - PE: scores (f32r, full precision) + Z accumulation (bf16) + the
   adjacency mask for a subset of j-tiles (fp8 240*I identity matmul
   into the scores PSUM; exp bias -270 zeroes masked lanes exactly).
 - ACT: only the 64 exps, writing e in bf16.
 - DVE: softmax-denominator accumulation as unit-wide [128,8192] bf16
   adds (2x_1p mode) + post-exp mask multiplies e *= adj for its share
   of j-tiles.
 - Pool (gpsimd): post-exp mask multiplies for its share.
j-units are rotated per core so each core's own query chunk of H^T
arrives first and stage 0 (qk) starts after ~0.5MiB of DMA.

Normalization is deferred through the MLP (relu commutes with positive
per-row scale): out = relu(relu(U@W1 + d*b1)@W2 + d*b2)/d, with the MLP
run transposed on stationary bf16 weights and 1/d applied by the final
per-tile ACT relu via the scale port.
"""
import numpy as np
import ml_dtypes
import concourse.bass as bass
from concourse import bacc
import concourse.mybir as mybir
from concourse.tile import TileContext
from concourse.bass_utils import run_bass_kernel_spmd

N = 8192
D = 128
NC = 8
RPC = N // NC          # rows per core = 1024
NU = 8                 # j units (1024 keys each)
TPU = 8                # j tiles per unit
F32 = mybir.dt.float32
F32R = mybir.dt.float32r
BF16 = mybir.dt.bfloat16
FP8 = mybir.dt.float8e4
MASK_D = 240.0         # fp8e4 max finite
STAB = 30.0            # global score shift, cancels in softmax

# mask owner per tile-index within a unit: 'P' = PE identity matmul,
# 'V' = DVE post-exp multiply, 'G' = Pool post-exp multiply
MASK_OWNER = ['P', 'P', 'V', 'V', 'V', 'G', 'G', 'G']

_CACHED = {}


def build(with_bias=False):
    nc = bacc.Bacc("TRN2", target_bir_lowering=False, debug=True)

    HT = [nc.dram_tensor(f"HT{u}", [D, 1024], F32R, kind="ExternalInput")
          for u in range(NU)]
    HN = [nc.dram_tensor(f"HN{u}", [D, TPU * D], BF16, kind="ExternalInput")
          for u in range(NU)]
    ADJ = [nc.dram_tensor(f"ADJ{u}", [D, TPU, RPC], FP8, kind="ExternalInput")
           for u in range(NU)]
    WQT = nc.dram_tensor("WQT", [D, D], F32R, kind="ExternalInput")
    WKT = nc.dram_tensor("WKT", [D, D], F32R, kind="ExternalInput")
    WVT = nc.dram_tensor("WVT", [D, D], F32R, kind="ExternalInput")
    W1 = nc.dram_tensor("W1", [D, D], F32R, kind="ExternalInput")
    W2 = nc.dram_tensor("W2", [D, D], BF16, kind="ExternalInput")
    B1R = nc.dram_tensor("B1R", [1, D], F32R, kind="ExternalInput")
    B2R = nc.dram_tensor("B2R", [1, D], F32R, kind="ExternalInput")
    I240 = nc.dram_tensor("I240", [D, D], FP8, kind="ExternalInput")
    ONES = nc.dram_tensor("ONES", [D, D], BF16, kind="ExternalInput")
    IDENT = nc.dram_tensor("IDENT", [D, D], F32, kind="ExternalInput")
    BIASM = nc.dram_tensor("BIASM", [D, 1], F32, kind="ExternalInput")
    BIASU = nc.dram_tensor("BIASU", [D, 1], F32, kind="ExternalInput")
    OUT = nc.dram_tensor("OUT", [RPC, D], F32, kind="ExternalOutput")

    with TileContext(nc) as tc:
        with (
            tc.tile_pool(name="pers", bufs=1) as pers,
            tc.tile_pool(name="adjp", bufs=3) as adjp,
            tc.tile_pool(name="ep", bufs=2) as ep,
            tc.tile_pool(name="psA", bufs=2, space="PSUM") as psA,   # [128,1024]
            tc.tile_pool(name="psB", bufs=2, space="PSUM") as psB,   # smalls
            tc.tile_pool(name="psZ", bufs=1, space="PSUM") as psZ,   # Z accumulator
        ):
            # ---- persistent tiles
            ht, hn = [], []
            for u in range(NU):
                ht_u = pers.tile([D, 1024], F32R, tag=f"ht{u}")
                ht.append(ht_u)
                hn_u = pers.tile([D, TPU * D], BF16, tag=f"hn{u}")
                hn.append(hn_u)
            wqt = pers.tile([D, D], F32R, tag="wqt")
            wkt = pers.tile([D, D], F32R, tag="wkt")
            wvt = pers.tile([D, D], F32R, tag="wvt")
            w1 = pers.tile([D, D], F32R, tag="w1")
            w2 = pers.tile([D, D], BF16, tag="w2")
            b1r = pers.tile([1, D], F32R, tag="b1r")
            b2r = pers.tile([1, D], F32R, tag="b2r")
            i240 = pers.tile([D, D], FP8, tag="i240")
            ones = pers.tile([D, D], BF16, tag="ones")
            ident = pers.tile([D, D], F32, tag="ident")
            biasm = pers.tile([D, 1], F32, tag="biasm")
            biasu = pers.tile([D, 1], F32, tag="biasu")

            # critical-path DMAs first (sync queue is in-order): own HT
            # chunk + first adj unit, then everything else interleaved.
            nc.sync.dma_start(out=wqt[:], in_=WQT[:])
            nc.sync.dma_start(out=wkt[:], in_=WKT[:])
            nc.sync.dma_start(out=ht[0][:], in_=HT[0][:])
            nc.sync.dma_start(out=biasm[:], in_=BIASM[:])
            nc.sync.dma_start(out=biasu[:], in_=BIASU[:])
            nc.sync.dma_start(out=i240[:], in_=I240[:])
            adj_sb = [None] * NU
            adj0 = adjp.tile([D, TPU, RPC], FP8, tag="adj")
            adj_sb[0] = adj0
            nc.sync.dma_start(out=adj_sb[0][:], in_=ADJ[0][:])
            nc.sync.dma_start(out=hn[0][:], in_=HN[0][:])
            for t, src in [(wvt, WVT), (w1, W1), (w2, W2), (ident, IDENT),
                           (b1r, B1R), (b2r, B2R), (ones, ONES)]:
                nc.sync.dma_start(out=t[:], in_=src[:])

            qk = pers.tile([D, RPC], F32R, tag="qk")
            mqk = pers.tile([D, D], F32R, tag="mqk")
            acc = pers.tile([D, TPU * RPC], BF16, tag="acc")
            zsb = pers.tile([D, RPC], BF16, tag="zsb")
            hts = pers.tile([D, RPC], BF16, tag="hts")    # hidden^T
            ots = pers.tile([D, RPC], F32, tag="ots")     # O'^T staging
            dentr = pers.tile([1, RPC], F32R, tag="dentr")
            dcol = pers.tile([D, NC], F32, tag="dcol")
            rcol = pers.tile([D, NC], F32, tag="rcol")
            w1v = pers.tile([D, D], BF16, tag="w1v")
            outsb = pers.tile([D, NC * D], F32, tag="outsb")

            # ---- stage 0: M = Wq @ Wk^T; qk[a,i] = sum_c M[c,a] * HT_own[c,i]
            mp = psB.tile([D, D], F32, tag="small")
            nc.tensor.matmul(mp[:], lhsT=wqt[:], rhs=wkt[:], start=True, stop=True)
            nc.scalar.copy(mqk[:], mp[:])
            ps2 = psA.tile([D, RPC], F32, tag="big")
            for h in range(2):
                cs = slice(h * 512, (h + 1) * 512)
                nc.tensor.matmul(ps2[:, cs], lhsT=mqk[:], rhs=ht[0][:, cs],
                                 start=True, stop=True)
            nc.scalar.copy(qk[:, 0:512], ps2[:, 0:512])
            nc.vector.tensor_copy(qk[:, 512:1024], ps2[:, 512:1024])
            wp = psB.tile([D, D], F32, tag="small")
            nc.tensor.matmul(wp[:], lhsT=wvt[:], rhs=w1[:], start=True, stop=True)
            nc.scalar.copy(w1v[:], wp[:])

            # ---- main loop over j units
            zps = psZ.tile([D, RPC], F32, tag="z")

            for u in range(NU):
                # prefetch next unit's tensors
                if u + 1 < NU:
                    adj_next = adjp.tile([D, TPU, RPC], FP8, tag="adj")
                    adj_sb[u + 1] = adj_next
                    nc.sync.dma_start(out=adj_sb[u + 1][:], in_=ADJ[u + 1][:])
                    nc.sync.dma_start(out=ht[u + 1][:], in_=HT[u + 1][:])
                    nc.sync.dma_start(out=hn[u + 1][:], in_=HN[u + 1][:])
                eg = ep.tile([D, TPU * RPC], BF16, tag="eg")
                for t in range(TPU):
                    jt = u * TPU + t
                    owner = MASK_OWNER[t]
                    sps = psA.tile([D, RPC], F32, tag="big")
                    if owner == 'P':
                        for h in range(2):
                            cs = slice(h * 512, (h + 1) * 512)
                            nc.tensor.matmul(sps[:, cs], lhsT=i240[:],
                                             rhs=adj_sb[u][:, t, cs],
                                             start=True, stop=False)
                    for h in range(2):
                        cs = slice(h * 512, (h + 1) * 512)
                        nc.tensor.matmul(sps[:, cs],
                                         lhsT=ht[u][:, t * D:(t + 1) * D],
                                         rhs=qk[:, cs],
                                         start=(owner != 'P'), stop=True)
                    eslot = eg[:, t * RPC:(t + 1) * RPC]
                    nc.scalar.activation(eslot, sps[:],
                                         mybir.ActivationFunctionType.Exp,
                                         bias=(biasm[:] if owner == 'P' else biasu[:]))
                    if owner == 'V':
                        nc.vector.tensor_mul(eslot, eslot, adj_sb[u][:, t, :])
                    elif owner == 'G':
                        nc.gpsimd.tensor_mul(eslot, eslot, adj_sb[u][:, t, :])
                    for h in range(2):
                        cs = slice(h * 512, (h + 1) * 512)
                        nc.tensor.matmul(zps[:, cs],
                                         lhsT=hn[u][:, t * D:(t + 1) * D],
                                         rhs=eg[:, t * RPC + h * 512:t * RPC + (h + 1) * 512],
                                         start=(jt == 0), stop=(jt == N // D - 1))
                # softmax-denominator accumulation, one unit-wide bf16 add
                with nc.allow_low_precision(reason="bf16 softmax denominator"):
                    if u == 0:
                        nc.vector.tensor_copy(acc[:], eg[:])
                    else:
                        nc.vector.tensor_add(acc[:], acc[:], eg[:])

            # ---- tail: denominator fold (slots+partitions) via
            # PSUM-accumulating ones-matmuls, then transposed MLP
            dps = psA.tile([D, RPC], F32, tag="big")
            for t in range(TPU):
                for h in range(2):
                    cs = slice(h * 512, (h + 1) * 512)
                    nc.tensor.matmul(
                        dps[:, cs], lhsT=ones[:],
                        rhs=acc[:, t * RPC + h * 512:t * RPC + (h + 1) * 512],
                        start=(t == 0), stop=(t == TPU - 1))
            nc.scalar.copy(dentr[:, 0:512], dps[0:1, 0:512])
            nc.vector.tensor_copy(dentr[:, 512:1024], dps[0:1, 512:1024])

            # MLP path (independent of d until the final scale)
            nc.vector.tensor_copy(zsb[:, 0:512], zps[:, 0:512])
            nc.scalar.copy(zsb[:, 512:1024], zps[:, 512:1024])
            gps = psA.tile([D, RPC], F32, tag="big")
            for h in range(2):
                cs = slice(h * 512, (h + 1) * 512)
                nc.tensor.matmul(gps[:, cs], lhsT=w1v[:], rhs=zsb[:, cs],
                                 start=True, stop=not with_bias)
                if with_bias:
                    nc.tensor.matmul(gps[:, cs], lhsT=b1r[:], rhs=dentr[:, cs],
                                     start=False, stop=True)
            # 1/denom columns: transposes slot in while ACT computes the relu
            rps = psB.tile([D, NC], F32, tag="small")
            for it in range(4):
                nc.tensor.transpose(rps[:, it:it + 1],
                                    dentr[0:1, it * 128:(it + 1) * 128].bitcast(F32),
                                    ident[0:1, 0:1])
            nc.scalar.activation(hts[:, 0:512], gps[:, 0:512],
                                 mybir.ActivationFunctionType.Relu)
            nc.vector.tensor_relu(hts[:, 512:1024], gps[:, 512:1024])
            ops_ = psA.tile([D, RPC], F32, tag="big")
            for h in range(2):
                cs = slice(h * 512, (h + 1) * 512)
                nc.tensor.matmul(ops_[:, cs], lhsT=w2[:], rhs=hts[:, cs],
                                 start=True, stop=not with_bias)
                if with_bias:
                    nc.tensor.matmul(ops_[:, cs], lhsT=b2r[:], rhs=dentr[:, cs],
                                     start=False, stop=True)
            for it in range(4, NC):
                nc.tensor.transpose(rps[:, it:it + 1],
                                    dentr[0:1, it * 128:(it + 1) * 128].bitcast(F32),
                                    ident[0:1, 0:1])
            nc.scalar.copy(dcol[:], rps[:])
            nc.vector.reciprocal(rcol[:], dcol[:])
            nc.scalar.copy(ots[:, 0:512], ops_[:, 0:512])
            nc.vector.tensor_copy(ots[:, 512:1024], ops_[:, 512:1024])
            for it in range(NC):
                tps = psB.tile([D, D], F32, tag="small")
                nc.tensor.transpose(tps[:], ots[:, it * 128:(it + 1) * 128], ident[:])
                nc.scalar.activation(outsb[:, it * 128:(it + 1) * 128], tps[:],
                                     mybir.ActivationFunctionType.Relu,
                                     scale=rcol[:, it:it + 1])

            outv = OUT.rearrange("(t p) d -> p t d", p=128)
            nc.sync.dma_start(out=outv[:, 0:4],
                              in_=outsb[:, 0:4 * D].rearrange("p (t d) -> p t d", t=4))
            nc.sync.dma_start(out=outv[:, 4:8],
                              in_=outsb[:, 4 * D:].rearrange("p (t d) -> p t d", t=4))
    nc.finalize()
    return nc


def _prep(H, adj, Wq, Wk, Wv, W1, b1, W2, b2):
    f8 = ml_dtypes.float8_e4m3
    bf = ml_dtypes.bfloat16
    H32 = np.asarray(H, dtype=np.float32)
    HT32 = np.ascontiguousarray(H32.T)                     # [D, N]
    # natural-row tiles per unit: HNP[u][p, t*D+ch] = H[u*1024 + t*128 + p, ch]
    HNP = H32.reshape(NU, TPU, D, D).transpose(0, 2, 1, 3).reshape(NU, D, TPU * D)
    adj = np.asarray(adj)
    base = {
        "WQT": np.ascontiguousarray(np.asarray(Wq, np.float32).T),
        "WKT": np.ascontiguousarray(np.asarray(Wk, np.float32).T),
        "WVT": np.ascontiguousarray(np.asarray(Wv, np.float32).T),
        "W1": np.asarray(W1, np.float32),
        "W2": np.asarray(W2, np.float32).astype(bf),
        "B1R": np.asarray(b1, np.float32).reshape(1, D),
        "B2R": np.asarray(b2, np.float32).reshape(1, D),
        "I240": (np.eye(D, dtype=np.float32) * MASK_D).astype(f8),
        "ONES": np.ones((D, D), np.float32).astype(bf),
        "IDENT": np.eye(D, dtype=np.float32),
        "BIASM": np.full((D, 1), -(MASK_D + STAB), np.float32),
        "BIASU": np.full((D, 1), -STAB, np.float32),
    }
    in_maps = []
    for c in range(NC):
        m = dict(base)
        # adj^T slice for this core, fp8, tiled [D, TPU, RPC] per unit
        adjTc = np.ascontiguousarray(
            adj[c * RPC:(c + 1) * RPC, :].T).astype(np.float32).astype(f8)
        adjTt = adjTc.reshape(NU, TPU, D, RPC)
        for u in range(NU):
            g = (c + u) % NU                                # rotated unit
            m[f"HT{u}"] = np.ascontiguousarray(HT32[:, g * 1024:(g + 1) * 1024])
            m[f"HN{u}"] = np.ascontiguousarray(HNP[g]).astype(bf)
            m[f"ADJ{u}"] = np.ascontiguousarray(adjTt[g].transpose(1, 0, 2))
        in_maps.append(m)
    return in_maps


def kernel(H, adj, Wq, Wk, Wv, W1, b1, W2, b2):
    wb = bool(np.any(np.asarray(b1)) or np.any(np.asarray(b2)))
    key = f"nc{int(wb)}"
    if key not in _CACHED:
        _CACHED[key] = build(with_bias=wb)
    in_maps = _prep(H, adj, Wq, Wk, Wv, W1, b1, W2, b2)
    res = run_bass_kernel_spmd(_CACHED[key], in_maps, list(range(NC)))
    return np.concatenate([res.results[c]["OUT"] for c in range(NC)], axis=0)
